# revision 30
# baseline (speedup 1.0000x reference)
"""Trainium2 Bass kernel for single-head attention (B=8, N=3136, C=147, D=64).

Sharding: data-parallel over batch across 8 NeuronCores (1 batch element/core).

Per-core algorithm (layouts chosen so the O(N^2) attention needs no transposes):
  Phase A: qkvT[j, n] = sum_c W_qkv[c, j] * x[n, c]
     - x tiles loaded natural [n,c], PE-transposed to xT [c, n] on chip
     - W_qkv q/k column blocks are duplicated so qT/kT land in BOTH partition
       halves of a [128, N] tile (enables PE row-group pairing below)
  Phase B: v_aug[j, 0:64] = v natural (PE transpose of vT), v_aug[j, 64] = 1.0
  Phase C: for each 512-wide i-chunk, for each pair of 128-wide j-tiles:
     S^T[j, i] = kT.T @ qT   -- TWO K=64 matmuls run concurrently in disjoint
                                PE row groups (base partitions 0 and 64)
     p = exp(S^T * scale)    -- one ACT call per pair ([128, 1024])
     o += v_aug.T @ p        -- K=128 split into two concurrent K=64 halves
                                accumulating separate psum tiles (o_a, o_b);
                                row 64 accumulates Z = sum_j p
     epilogue: stU = o_a + o_b; proj (normalization commutes with the linear
     proj); small PE transposes to natural layout; out = pj*(1/Z) + v + b.
Matmuls run in float32r (fp32 storage, ~tf32 precision, 1 cycle/row on PE).
"""
import sys

for _p in ("/opt/trn_rl_repo",):
    if _p not in sys.path:
        sys.path.append(_p)

import numpy as np
from contextlib import ExitStack

import concourse.bass as bass
import concourse.bacc as bacc
import concourse.tile as tile
from concourse import mybir
from concourse.bass_utils import run_bass_kernel_spmd
from concourse.masks import make_identity

P = 128
SEQ = 3136        # N
CH = 147          # C
D = 64            # head dim
SCALE = D ** -0.5
NT = (SEQ + P - 1) // P          # 25 tiles of n/j (24 full + 1 of 64)
IC = 512                         # i-chunk width for attention
F32 = mybir.dt.float32
F32R = mybir.dt.float32r
BF = mybir.dt.bfloat16
EXP = mybir.ActivationFunctionType.Exp

_cache = {}


def _ichunks():
    out = []
    i0 = 0
    while i0 < SEQ:
        out.append((i0, min(IC, SEQ - i0)))
        i0 += IC
    return out


def build():
    nc = bacc.Bacc("TRN2", target_bir_lowering=False, debug=False, num_devices=8)
    x = nc.declare_dram_parameter("x", [SEQ, CH], F32, isOutput=False)
    w_qkv = nc.declare_dram_parameter("w_qkv", [CH, 3 * D], F32, isOutput=False)
    w_proj = nc.declare_dram_parameter("w_proj", [D, D], F32, isOutput=False)
    b_proj = nc.declare_dram_parameter("b_proj", [D], F32, isOutput=False)
    out = nc.declare_dram_parameter("out", [SEQ, D], F32, isOutput=True)

    with ExitStack() as ctx:
        tc = ctx.enter_context(tile.TileContext(nc))
        singles = ctx.enter_context(tc.tile_pool(name="singles", bufs=1))

        ident = singles.tile([P, P], F32)
        make_identity(nc, ident)
        ident_bf = singles.tile([P, P], BF)
        nc.vector.tensor_copy(ident_bf, ident)

        # --- weights ---
        w_hi = singles.tile([P, 3 * D], F32)
        w_lo = singles.tile([CH - P, 3 * D], F32)
        nc.sync.dma_start(out=w_hi, in_=w_qkv[0:P, :])
        nc.sync.dma_start(out=w_lo, in_=w_qkv[P:CH, :])
        # duplicated q/k blocks: [Wq | Wq], [Wk | Wk]; v block plain
        wq2_hi = singles.tile([P, P], BF)
        wq2_lo = singles.tile([CH - P, P], BF)
        wk2_hi = singles.tile([P, P], BF)
        wk2_lo = singles.tile([CH - P, P], BF)
        wv_hi = singles.tile([P, D], BF)
        wv_lo = singles.tile([CH - P, D], BF)
        for half in (0, 1):
            nc.vector.tensor_copy(wq2_hi[:, half * D:half * D + D], w_hi[:, 0:D])
            nc.vector.tensor_copy(wq2_lo[:, half * D:half * D + D], w_lo[:, 0:D])
            nc.vector.tensor_copy(wk2_hi[:, half * D:half * D + D], w_hi[:, D:2 * D])
            nc.vector.tensor_copy(wk2_lo[:, half * D:half * D + D], w_lo[:, D:2 * D])
        nc.vector.tensor_copy(wv_hi, w_hi[:, 2 * D:3 * D])
        nc.vector.tensor_copy(wv_lo, w_lo[:, 2 * D:3 * D])

        wp = singles.tile([D, D], F32)
        nc.sync.dma_start(out=wp, in_=w_proj[:, :])
        wp_r = singles.tile([D, D], F32R)
        nc.vector.tensor_copy(wp_r, wp)

        # b_proj broadcast across partitions: bb[p, d] = b_proj[d]
        bb = singles.tile([P, D], F32)
        bp_ap = b_proj.ap()
        bb_src = bass.AP(tensor=bp_ap.tensor, offset=bp_ap.offset,
                         ap=[[0, P]] + list(bp_ap.ap))
        nc.sync.dma_start(out=bb, in_=bb_src)

        # ones column (Z-row transpose rhs at base partition 64, v_aug fill)
        ones_t = singles.tile([P, 1], F32)
        nc.vector.memset(ones_t, 1.0)

        # --- big SBUF holdings ---
        qT2 = singles.tile([P, SEQ], F32R)        # qT duplicated in both halves
        kT2 = singles.tile([P, SEQ], F32R)        # kT duplicated in both halves
        vT = singles.tile([D, SEQ], F32)          # vT[d, n]
        v_aug = singles.tile([P, NT, D + 1], BF)  # v natural + ones col (PV lhsT)
        v_nat32 = singles.tile([P, NT, D], F32)   # v natural, fp32 (residual)

        # ---------------- Phase A: qkvT ----------------
        # x tiles are PE-transposed (fp32 transpose mode) and cast to bf16 on
        # the way out of PSUM; qkv matmuls run in bf16, S^T stays f32r.
        with ExitStack() as actx:
            a_sb = actx.enter_context(tc.tile_pool(name="a_sb", bufs=4))
            a_xt = actx.enter_context(tc.tile_pool(name="a_xt", bufs=3))
            a_ps = actx.enter_context(tc.tile_pool(name="a_ps", bufs=2, space="PSUM"))
            a_mm = actx.enter_context(tc.tile_pool(name="a_mm", bufs=1, space="PSUM"))

            def emit_transposes(n0, csz):
                nsub = (csz + P - 1) // P
                tp_hi = a_ps.tile([P, 512], F32, name="tp_hi")
                tp_lo = a_ps.tile([32, 512], F32, name="tp_lo")
                for s in range(nsub):
                    ssz = min(P, csz - s * P)
                    x_t = a_sb.tile([P, CH], F32, name="x_t")
                    nc.sync.dma_start(out=x_t[0:ssz, :],
                                      in_=x[n0 + s * P:n0 + s * P + ssz, :])
                    nc.tensor.transpose(
                        tp_hi[:, s * P:s * P + ssz], x_t[0:ssz, 0:P],
                        ident[0:ssz, 0:ssz])
                    nc.tensor.transpose(
                        tp_lo[0:CH - P, s * P:s * P + ssz], x_t[0:ssz, P:CH],
                        ident[0:ssz, 0:ssz])
                xt_hi = a_xt.tile([P, 512], BF, name="xt_hi")
                xt_lo = a_xt.tile([32, 512], BF, name="xt_lo")
                nc.vector.tensor_copy(xt_hi[:, 0:csz], tp_hi[:, 0:csz])
                nc.vector.tensor_copy(xt_lo[0:CH - P, 0:csz],
                                      tp_lo[0:CH - P, 0:csz])
                return xt_hi, xt_lo

            def emit_qkv(n0, csz, xt_hi, xt_lo):
                pq = a_mm.tile([P, 512], F32, name="pq")
                pk = a_mm.tile([P, 512], F32, name="pk")
                pv = a_mm.tile([D, 512], F32, name="pv")
                for (ps_t, whi, wlo) in ((pq, wq2_hi, wq2_lo),
                                         (pk, wk2_hi, wk2_lo),
                                         (pv, wv_hi, wv_lo)):
                    nc.tensor.matmul(ps_t[:, 0:csz], whi, xt_hi[:, 0:csz],
                                     start=True, stop=False)
                    nc.tensor.matmul(ps_t[:, 0:csz], wlo[0:CH - P, :],
                                     xt_lo[0:CH - P, 0:csz],
                                     start=False, stop=True)
                nc.vector.tensor_copy(qT2[:, n0:n0 + csz], pq[:, 0:csz])
                nc.vector.tensor_copy(kT2[:, n0:n0 + csz], pk[:, 0:csz])
                nc.vector.tensor_copy(vT[:, n0:n0 + csz], pv[:, 0:csz])

            chunks = []
            n0 = 0
            while n0 < SEQ:
                chunks.append((n0, min(512, SEQ - n0)))
                n0 += 512
            pending = None   # (n0, csz, xt_hi, xt_lo)
            for (n0, csz) in chunks:
                xt = emit_transposes(n0, csz)
                if pending is not None:
                    emit_qkv(pending[0], pending[1], pending[2], pending[3])
                pending = (n0, csz, xt[0], xt[1])
            emit_qkv(*pending)

        # ---------------- Phase B: v natural + ones ----------------
        with ExitStack() as bctx:
            b_ps = bctx.enter_context(tc.tile_pool(name="b_ps", bufs=2, space="PSUM"))
            for jt in range(NT):
                jsz = min(P, SEQ - jt * P)
                tv = b_ps.tile([P, D], F32, name="tv")
                nc.tensor.transpose(
                    tv[0:jsz, :], vT[:, jt * P:jt * P + jsz], ident[0:D, 0:D])
                nc.vector.tensor_copy(v_aug[0:jsz, jt, 0:D], tv[0:jsz, :])
                nc.vector.tensor_copy(v_nat32[0:jsz, jt, :], tv[0:jsz, :])
                nc.vector.tensor_copy(v_aug[0:jsz, jt, D:D + 1], ones_t[0:jsz, :])

        # ---------------- Phase C: attention ----------------
        with ExitStack() as cctx:
            st_ps = cctx.enter_context(tc.tile_pool(name="st_ps", bufs=2, space="PSUM"))
            o_ps_pool = cctx.enter_context(tc.tile_pool(name="o_ps", bufs=2, space="PSUM"))
            ot_ps_pool = cctx.enter_context(tc.tile_pool(name="ot_ps", bufs=2, space="PSUM"))
            p_pool = cctx.enter_context(tc.tile_pool(name="p_sb", bufs=4))
            e_sb = cctx.enter_context(tc.tile_pool(name="e_sb", bufs=2))
            o_sb = cctx.enter_context(tc.tile_pool(name="o_sb", bufs=4))

            npairs = (NT + 1) // 2    # 13: 12 full pairs + 1 single

            def emit_pv(o_ps, p, pt, icsz):
                jtA, jtB = 2 * pt, 2 * pt + 1
                if jtB < NT:
                    nc.tensor.matmul(o_ps, v_aug[:, jtA, :], p[:, 0, 0:icsz],
                                     start=(jtA == 0), stop=False)
                    nc.tensor.matmul(o_ps, v_aug[:, jtB, :], p[:, 1, 0:icsz],
                                     start=False, stop=(jtB == NT - 1))
                else:
                    jsz = SEQ - jtA * P   # 64
                    nc.tensor.matmul(o_ps, v_aug[0:jsz, jtA, :],
                                     p[0:jsz, 0, 0:icsz],
                                     start=False, stop=True)

            def emit_epilogue(o_ps, i0, icsz):
                stU = e_sb.tile([D + 1, IC], F32R, name="stU")[:, 0:icsz]
                nc.vector.tensor_copy(stU, o_ps)
                pj = o_ps_pool.tile([D, IC], F32, tag="oa", name="pj")[:, 0:icsz]
                nc.tensor.matmul(pj, wp_r, stU[0:D, :], start=True, stop=True)
                pjs = e_sb.tile([D, IC], F32, name="pjs")[:, 0:icsz]
                nc.vector.tensor_copy(pjs, pj)
                for t in range((icsz + P - 1) // P):
                    ncols = min(P, icsz - t * P)
                    nt_idx = (i0 + t * P) // P
                    ot = ot_ps_pool.tile([P, D + 1], F32, name="ot")
                    nc.tensor.transpose(
                        ot[0:ncols, 0:D], pjs[:, t * P:t * P + ncols],
                        ident[0:D, 0:D])
                    nc.tensor.transpose(
                        ot[0:ncols, D:D + 1],
                        stU[D:D + 1, t * P:t * P + ncols].bitcast(F32),
                        ones_t[D:D + 1, :])
                    rz = o_sb.tile([P, 1], F32, name="rz")
                    nc.vector.reciprocal(rz[0:ncols, :], ot[0:ncols, D:D + 1])
                    res = o_sb.tile([P, D], F32, name="res")
                    nc.vector.scalar_tensor_tensor(
                        res[0:ncols, :],
                        ot[0:ncols, 0:D],
                        rz[0:ncols, :],
                        v_nat32[0:ncols, nt_idx, :],
                        op0=mybir.AluOpType.mult,
                        op1=mybir.AluOpType.add)
                    nc.vector.tensor_add(res[0:ncols, :], res[0:ncols, :],
                                         bb[0:ncols, :])
                    nc.sync.dma_start(
                        out=out[i0 + t * P:i0 + t * P + ncols, :],
                        in_=res[0:ncols, :])

            # Software-pipelined: PV trails S^T/exp by one pair so the in-order
            # PE never stalls waiting for exp; the epilogue trails by one chunk.
            pending_epi = None       # (o_ps, i0, icsz) of previous i-chunk
            for (i0, icsz) in _ichunks():
                o_ps = o_ps_pool.tile([D + 1, IC], F32, tag="oa", name="o_ps")[:, 0:icsz]
                pending_pv = None    # (p, pt)
                for pt in range(npairs):
                    jtA, jtB = 2 * pt, 2 * pt + 1
                    pair = jtB < NT
                    st = st_ps.tile([P, 2, IC], F32, name="st")
                    p = p_pool.tile([P, 2, IC], BF, name="p")
                    jwA = min(P, SEQ - jtA * P)
                    nc.tensor.matmul(
                        st[0:jwA, 0, 0:icsz],
                        kT2[0:D, jtA * P:jtA * P + jwA],
                        qT2[0:D, i0:i0 + icsz],
                        start=True, stop=True)
                    if pair:
                        nc.tensor.matmul(
                            st[:, 1, 0:icsz],
                            kT2[D:P, jtB * P:(jtB + 1) * P],
                            qT2[D:P, i0:i0 + icsz],
                            start=True, stop=True)
                        nc.scalar.activation(p[:, :, 0:icsz], st[:, :, 0:icsz],
                                             EXP, scale=SCALE)
                    else:
                        jsz = SEQ - jtA * P
                        nc.scalar.activation(p[0:jsz, 0, 0:icsz],
                                             st[0:jsz, 0, 0:icsz],
                                             EXP, scale=SCALE)
                    if pending_pv is not None:
                        emit_pv(o_ps, pending_pv[0], pending_pv[1], icsz)
                    pending_pv = (p, pt)
                    if pt == 0 and pending_epi is not None:
                        emit_epilogue(*pending_epi)
                        pending_epi = None
                emit_pv(o_ps, pending_pv[0], pending_pv[1], icsz)
                pending_epi = (o_ps, i0, icsz)
            emit_epilogue(*pending_epi)

    nc.compile()
    return nc


def kernel(x, W_qkv, W_proj, b_proj):
    B = x.shape[0]
    if "nc" not in _cache:
        _cache["nc"] = build()
    nc = _cache["nc"]
    in_maps = [
        {
            "x": np.ascontiguousarray(x[b], dtype=np.float32),
            "w_qkv": np.ascontiguousarray(W_qkv, dtype=np.float32),
            "w_proj": np.ascontiguousarray(W_proj, dtype=np.float32),
            "b_proj": np.ascontiguousarray(b_proj, dtype=np.float32),
        }
        for b in range(B)
    ]
    res = run_bass_kernel_spmd(nc, in_maps, core_ids=list(range(B)))
    return np.stack([res.results[b]["out"] for b in range(B)], axis=0)


if __name__ == "__main__":
    rng = np.random.default_rng(0)
    x = rng.standard_normal((8, SEQ, CH), dtype=np.float32)
    W_qkv = (rng.standard_normal((CH, 3 * D), dtype=np.float32) * CH ** -0.5)
    W_proj = (rng.standard_normal((D, D), dtype=np.float32) * D ** -0.5)
    b_proj = np.zeros(D, dtype=np.float32)
    out = kernel(x, W_qkv, W_proj, b_proj)
    print("out", out.shape, out.dtype)
